# revision 11
# baseline (speedup 1.0000x reference)
"""Fused RNN cell on 8 Trainium2 NeuronCores.

Reference computation (fp32):
    combined   = [x, hidden]                      [B=4096, I+H=4096]
    new_hidden = tanh(combined @ W_ih^T + b_ih)   [B, H=2048]
    output     = new_hidden @ W_ho^T + b_ho       [B, O=2048]
    returns (output, new_hidden)

Strategy: data-parallel over the batch — each of the 8 cores processes 512
batch rows with replicated weights; no collectives. All operand layout
transforms (transposes into PE-friendly [K-partition, free] form) happen on
the host so every device DMA is a fat, fully contiguous transfer:

    c   [128, 32, 512]      cL[ki, ko, b]       = combined[b, ko*128+ki]
    w1  [128, 32, 16, 128]  w1L[ki, ko, hc, h]  = W_ih[hc*128+h, ko*128+ki]
    w2  [128, 16, 16, 128]  w2L[hi, ho, oc, o]  = W_ho[oc*128+o, ho*128+hi]
    b1  [128, 16]           b1L[p, hc]          = b_ih[hc*128+p]
    b2  [128, 16]           b2L[p, oc]          = b_ho[oc*128+p]

Matmuls run in fp32r (full-rate fp32 on the PE; TF32-like rounding).
mm1 produces nh^T [h, b] tiles in SBUF, which feed mm2 directly as the
stationary operand; mm2 produces out^T [o, b]. Both outputs are written
transposed and un-transposed on the host after the gather.

Loop structure: h-chunks (and o-chunks) are processed in groups of 8, one
PSUM bank per chunk. Each inner step streams one [128, 8, 128] weight
slice (512 KB) and, in the first group, one [128, 512] c-chunk (256 KB),
then issues 8 accumulating matmuls — so the PE starts ~2 µs after launch
and DMA/compute stay interleaved at ~0.5 MiB granularity throughout.
"""

import numpy as np

import concourse.bass as bass
import concourse.mybir as mybir
import concourse.tile as tile
from concourse import bacc, bass_utils

NCORES = 8
B, I, H, O = 4096, 2048, 2048, 2048
BC = B // NCORES          # 512 batch rows per core
K1 = I + H                # mm1 contraction dim, 4096
KO1 = K1 // 128           # 32 k-chunks for mm1
HC = H // 128             # 16 h-chunks
OC = O // 128             # 16 o-chunks
G = 4                     # h/o-chunks per PSUM group (ping-pong over 8 banks)
P = 128
F32 = mybir.dt.float32
F32R = mybir.dt.float32r
AF = mybir.ActivationFunctionType


def _build():
    nc = bacc.Bacc("TRN2", target_bir_lowering=False)

    c = nc.dram_tensor("c", [P, KO1, BC], F32R, kind="ExternalInput")
    w1 = nc.dram_tensor("w1", [P, KO1, HC, P], F32R, kind="ExternalInput")
    b1 = nc.dram_tensor("b1", [P, HC], F32, kind="ExternalInput")
    w2 = nc.dram_tensor("w2", [P, HC, OC, P], F32R, kind="ExternalInput")
    b2 = nc.dram_tensor("b2", [P, OC], F32, kind="ExternalInput")
    nhT = nc.dram_tensor("nhT", [H, BC], F32R, kind="ExternalOutput")
    outT = nc.dram_tensor("outT", [O, BC], F32, kind="ExternalOutput")

    with tile.TileContext(nc) as tc:
        with tc.tile_pool(name="cpool", bufs=1) as cpool, \
             tc.tile_pool(name="wpool", bufs=6) as wpool, \
             tc.tile_pool(name="nhpool", bufs=1) as nhpool, \
             tc.tile_pool(name="opool", bufs=8) as opool, \
             tc.tile_pool(name="bpool", bufs=1) as bpool, \
             tc.tile_pool(name="ps", bufs=8, space="PSUM") as ps:

            b1_sb = bpool.tile([P, HC], F32)
            b2_sb = bpool.tile([P, OC], F32)
            # Biases aren't needed until the first group drains; keep them
            # off the sync HWDGE ring entirely (SWDGE via GpSimd).
            nc.gpsimd.dma_start(b1_sb[:], b1[:])
            nc.gpsimd.dma_start(b2_sb[:], b2[:])

            c_sb = cpool.tile([P, KO1, BC], F32R)
            nh_sb = nhpool.tile([P, HC, BC], F32R)

            # PE warm-up: the HAM clock gate holds the PE at 1.2 GHz until
            # it has been busy ~3.4 µs. Dummy matmuls (no data deps beyond
            # one memset) keep the PE active while the first input tiles
            # stream in, so real matmuls start at 2.4 GHz.
            warm_sb = bpool.tile([P, P], mybir.dt.bfloat16)
            nc.vector.memset(warm_sb[:], 0.0)
            warm_ps = ps.tile([P, BC], F32, tag="ps", name="warm")
            for _ in range(70):
                nc.tensor.matmul(
                    warm_ps[:, :P], lhsT=warm_sb[:], rhs=warm_sb[:],
                    start=True, stop=True, skip_group_check=True,
                )

            # Stores are deferred one group: group g's stores are emitted
            # after group g+1's loads, so when the sync sequencer reaches
            # them the producing compute finished long ago and the ring
            # never head-of-line blocks on a store waiting for compute.
            deferred = []

            def flush_deferred():
                for fn in deferred:
                    fn()
                deferred.clear()

            # mm1: nh^T[h, b] = tanh(W_ih @ combined^T + b_ih)
            # G-sized PSUM groups ping-pong across the 8 banks: while one
            # group's banks drain through ACT, the next group accumulates
            # into the other four — group boundaries cost the PE nothing.
            for g in range(HC // G):
                psums = [ps.tile([P, BC], F32, tag="ps", name=f"ps{i}")
                         for i in range(G)]
                for ko2 in range(KO1 // 2):
                    ko0 = 2 * ko2
                    if g == 0:
                        nc.sync.dma_start(c_sb[:, ko0:ko0 + 2], c[:, ko0:ko0 + 2])
                    w1_sb = wpool.tile([P, 2, G, P], F32R, tag="w")
                    nc.sync.dma_start(
                        w1_sb[:], w1[:, ko0:ko0 + 2, g * G:(g + 1) * G])
                    for kk in range(2):
                        for i in range(G):
                            nc.tensor.matmul(
                                psums[i][:],
                                lhsT=w1_sb[:, kk, i],
                                rhs=c_sb[:, ko0 + kk],
                                start=(ko0 + kk == 0),
                                stop=(ko0 + kk == KO1 - 1),
                            )
                flush_deferred()
                for i in range(G):
                    hc = g * G + i
                    nc.scalar.activation(
                        nh_sb[:, hc], psums[i][:], AF.Tanh,
                        bias=b1_sb[:, hc:hc + 1],
                    )
                    deferred.append(
                        lambda hc=hc: nc.sync.dma_start(
                            nhT[hc * P:(hc + 1) * P, :], nh_sb[:, hc])
                    )

            # mm2: out^T[o, b] = W_ho @ nh^T + b_ho
            for g in range(OC // G):
                psums = [ps.tile([P, BC], F32, tag="ps", name=f"ps{i}")
                         for i in range(G)]
                for ho2 in range(HC // 2):
                    ho0 = 2 * ho2
                    w2_sb = wpool.tile([P, 2, G, P], F32R, tag="w")
                    nc.sync.dma_start(
                        w2_sb[:], w2[:, ho0:ho0 + 2, g * G:(g + 1) * G])
                    for kk in range(2):
                        for i in range(G):
                            nc.tensor.matmul(
                                psums[i][:],
                                lhsT=w2_sb[:, kk, i],
                                rhs=nh_sb[:, ho0 + kk],
                                start=(ho0 + kk == 0),
                                stop=(ho0 + kk == HC - 1),
                            )
                last = g == OC // G - 1
                flush_deferred()
                for i in range(G):
                    oc = g * G + i
                    o_sb = opool.tile([P, BC], F32, tag="osb")
                    nc.vector.tensor_tensor(
                        o_sb[:],
                        psums[i][:],
                        b2_sb[:, oc:oc + 1].to_broadcast([P, BC]),
                        mybir.AluOpType.add,
                    )
                    # Final group: split stores across both HWDGE rings so
                    # the tail drains in parallel.
                    eng = (nc.scalar if (last and i % 2) else nc.sync)
                    deferred.append(
                        lambda oc=oc, o_sb=o_sb, eng=eng: eng.dma_start(
                            outT[oc * P:(oc + 1) * P, :], o_sb[:])
                    )
            flush_deferred()

    nc.compile()
    return nc


def _shard_inputs(x, hidden, W_ih, b_ih, W_ho, b_ho):
    combined = np.concatenate([x, hidden], axis=1)  # [B, K1]
    w1L = np.ascontiguousarray(
        W_ih.reshape(HC, P, KO1, P).transpose(3, 2, 0, 1)
    )  # [ki, ko, hc, h]
    w2L = np.ascontiguousarray(
        W_ho.reshape(OC, P, HC, P).transpose(3, 2, 0, 1)
    )  # [hi, ho, oc, o]
    b1L = np.ascontiguousarray(b_ih.reshape(HC, P).T)
    b2L = np.ascontiguousarray(b_ho.reshape(OC, P).T)
    in_maps = []
    for cix in range(NCORES):
        cc = combined[cix * BC:(cix + 1) * BC]  # [BC, K1]
        cL = np.ascontiguousarray(cc.reshape(BC, KO1, P).transpose(2, 1, 0))
        in_maps.append(
            {"c": cL, "w1": w1L, "b1": b1L, "w2": w2L, "b2": b2L}
        )
    return in_maps


def _run(in_maps, **kwargs):
    nc = _build()
    return bass_utils.run_bass_kernel_spmd(
        nc, in_maps, core_ids=list(range(NCORES)), **kwargs
    )


def kernel(x, hidden, W_ih, b_ih, W_ho, b_ho):
    x = np.asarray(x, dtype=np.float32)
    hidden = np.asarray(hidden, dtype=np.float32)
    W_ih = np.asarray(W_ih, dtype=np.float32)
    b_ih = np.asarray(b_ih, dtype=np.float32)
    W_ho = np.asarray(W_ho, dtype=np.float32)
    b_ho = np.asarray(b_ho, dtype=np.float32)

    in_maps = _shard_inputs(x, hidden, W_ih, b_ih, W_ho, b_ho)
    res = _run(in_maps)
    output = np.concatenate([r["outT"].T for r in res.results], axis=0)
    new_hidden = np.concatenate([r["nhT"].T for r in res.results], axis=0)
    return output, new_hidden


# revision 12
# speedup vs baseline: 1.0597x; 1.0597x over previous
"""Fused RNN cell on 8 Trainium2 NeuronCores.

Reference computation (fp32):
    combined   = [x, hidden]                      [B=4096, I+H=4096]
    new_hidden = tanh(combined @ W_ih^T + b_ih)   [B, H=2048]
    output     = new_hidden @ W_ho^T + b_ho       [B, O=2048]
    returns (output, new_hidden)

Strategy: data-parallel over the batch — each of the 8 cores processes 512
batch rows with replicated weights; no collectives. All operand layout
transforms (transposes into PE-friendly [K-partition, free] form) happen on
the host so every device DMA is a fat, fully contiguous transfer:

    c   [128, 32, 512]      cL[ki, ko, b]       = combined[b, ko*128+ki]
    w1  [128, 32, 16, 128]  w1L[ki, ko, hc, h]  = W_ih[hc*128+h, ko*128+ki]
    w2  [128, 16, 16, 128]  w2L[hi, ho, oc, o]  = W_ho[oc*128+o, ho*128+hi]
    b1  [128, 16]           b1L[p, hc]          = b_ih[hc*128+p]
    b2  [128, 16]           b2L[p, oc]          = b_ho[oc*128+p]

Matmuls run in fp32r (full-rate fp32 on the PE; TF32-like rounding).
mm1 produces nh^T [h, b] tiles in SBUF, which feed mm2 directly as the
stationary operand; mm2 produces out^T [o, b]. Both outputs are written
transposed and un-transposed on the host after the gather.

Loop structure: h-chunks (and o-chunks) are processed in groups of 8, one
PSUM bank per chunk. Each inner step streams one [128, 8, 128] weight
slice (512 KB) and, in the first group, one [128, 512] c-chunk (256 KB),
then issues 8 accumulating matmuls — so the PE starts ~2 µs after launch
and DMA/compute stay interleaved at ~0.5 MiB granularity throughout.
"""

import numpy as np

import concourse.bass as bass
import concourse.mybir as mybir
import concourse.tile as tile
from concourse import bacc, bass_utils

NCORES = 8
B, I, H, O = 4096, 2048, 2048, 2048
BC = B // NCORES          # 512 batch rows per core
K1 = I + H                # mm1 contraction dim, 4096
KO1 = K1 // 128           # 32 k-chunks for mm1
HC = H // 128             # 16 h-chunks
OC = O // 128             # 16 o-chunks
G = 8                     # h/o-chunks per PSUM group (8 banks)
P = 128
F32 = mybir.dt.float32
F32R = mybir.dt.float32r
AF = mybir.ActivationFunctionType


def _build():
    nc = bacc.Bacc("TRN2", target_bir_lowering=False)

    c = nc.dram_tensor("c", [P, KO1, BC], F32R, kind="ExternalInput")
    w1 = nc.dram_tensor("w1", [P, KO1, HC, P], F32R, kind="ExternalInput")
    b1 = nc.dram_tensor("b1", [P, HC], F32, kind="ExternalInput")
    w2 = nc.dram_tensor("w2", [P, HC, OC, P], F32R, kind="ExternalInput")
    b2 = nc.dram_tensor("b2", [P, OC], F32, kind="ExternalInput")
    nhT = nc.dram_tensor("nhT", [H, BC], F32R, kind="ExternalOutput")
    outT = nc.dram_tensor("outT", [O, BC], F32, kind="ExternalOutput")

    with tile.TileContext(nc) as tc:
        with tc.tile_pool(name="cpool", bufs=1) as cpool, \
             tc.tile_pool(name="wpool", bufs=6) as wpool, \
             tc.tile_pool(name="nhpool", bufs=1) as nhpool, \
             tc.tile_pool(name="opool", bufs=8) as opool, \
             tc.tile_pool(name="bpool", bufs=1) as bpool, \
             tc.tile_pool(name="ps", bufs=8, space="PSUM") as ps:

            b1_sb = bpool.tile([P, HC], F32)
            b2_sb = bpool.tile([P, OC], F32)
            # Biases aren't needed until the first group drains; keep them
            # off the sync HWDGE ring entirely (SWDGE via GpSimd).
            nc.gpsimd.dma_start(b1_sb[:], b1[:])
            nc.gpsimd.dma_start(b2_sb[:], b2[:])

            c_sb = cpool.tile([P, KO1, BC], F32R)
            nh_sb = nhpool.tile([P, HC, BC], F32R)

            # PE warm-up: the HAM clock gate holds the PE at 1.2 GHz until
            # it has been busy ~3.4 µs. Dummy matmuls (no data deps beyond
            # one memset) keep the PE active while the first input tiles
            # stream in, so real matmuls start at 2.4 GHz.
            warm_sb = bpool.tile([P, P], mybir.dt.bfloat16)
            nc.vector.memset(warm_sb[:], 0.0)
            warm_ps = ps.tile([P, BC], F32, tag="ps", name="warm")
            for _ in range(70):
                nc.tensor.matmul(
                    warm_ps[:, :P], lhsT=warm_sb[:], rhs=warm_sb[:],
                    start=True, stop=True, skip_group_check=True,
                )

            # Stores are deferred one group: group g's stores are emitted
            # after group g+1's loads, so when the sync sequencer reaches
            # them the producing compute finished long ago and the ring
            # never head-of-line blocks on a store waiting for compute.
            deferred = []

            def flush_deferred():
                for fn in deferred:
                    fn()
                deferred.clear()

            # mm1: nh^T[h, b] = tanh(W_ih @ combined^T + b_ih)
            # G-sized PSUM groups ping-pong across the 8 banks: while one
            # group's banks drain through ACT, the next group accumulates
            # into the other four — group boundaries cost the PE nothing.
            for g in range(HC // G):
                psums = [ps.tile([P, BC], F32, tag="ps", name=f"ps{i}")
                         for i in range(G)]
                for ko in range(KO1):
                    if g == 0:
                        nc.sync.dma_start(c_sb[:, ko], c[:, ko])
                    w1_sb = wpool.tile([P, G, P], F32R, tag="w")
                    nc.sync.dma_start(w1_sb[:], w1[:, ko, g * G:(g + 1) * G])
                    for i in range(G):
                        nc.tensor.matmul(
                            psums[i][:],
                            lhsT=w1_sb[:, i],
                            rhs=c_sb[:, ko],
                            start=(ko == 0),
                            stop=(ko == KO1 - 1),
                        )
                flush_deferred()
                for i in range(G):
                    hc = g * G + i
                    nc.scalar.activation(
                        nh_sb[:, hc], psums[i][:], AF.Tanh,
                        bias=b1_sb[:, hc:hc + 1],
                    )
                    deferred.append(
                        lambda hc=hc: nc.sync.dma_start(
                            nhT[hc * P:(hc + 1) * P, :], nh_sb[:, hc])
                    )

            # mm2: out^T[o, b] = W_ho @ nh^T + b_ho
            for g in range(OC // G):
                psums = [ps.tile([P, BC], F32, tag="ps", name=f"ps{i}")
                         for i in range(G)]
                for ho in range(HC):
                    w2_sb = wpool.tile([P, G, P], F32R, tag="w")
                    nc.sync.dma_start(w2_sb[:], w2[:, ho, g * G:(g + 1) * G])
                    for i in range(G):
                        nc.tensor.matmul(
                            psums[i][:],
                            lhsT=w2_sb[:, i],
                            rhs=nh_sb[:, ho],
                            start=(ho == 0),
                            stop=(ho == HC - 1),
                        )
                last = g == OC // G - 1
                flush_deferred()
                for i in range(G):
                    oc = g * G + i
                    o_sb = opool.tile([P, BC], F32, tag="osb")
                    nc.vector.tensor_tensor(
                        o_sb[:],
                        psums[i][:],
                        b2_sb[:, oc:oc + 1].to_broadcast([P, BC]),
                        mybir.AluOpType.add,
                    )
                    # Final group: split stores across both HWDGE rings so
                    # the tail drains in parallel.
                    eng = (nc.scalar if (last and i % 2) else nc.sync)
                    deferred.append(
                        lambda oc=oc, o_sb=o_sb, eng=eng: eng.dma_start(
                            outT[oc * P:(oc + 1) * P, :], o_sb[:])
                    )
            flush_deferred()

    nc.compile()
    return nc


def _shard_inputs(x, hidden, W_ih, b_ih, W_ho, b_ho):
    combined = np.concatenate([x, hidden], axis=1)  # [B, K1]
    w1L = np.ascontiguousarray(
        W_ih.reshape(HC, P, KO1, P).transpose(3, 2, 0, 1)
    )  # [ki, ko, hc, h]
    w2L = np.ascontiguousarray(
        W_ho.reshape(OC, P, HC, P).transpose(3, 2, 0, 1)
    )  # [hi, ho, oc, o]
    b1L = np.ascontiguousarray(b_ih.reshape(HC, P).T)
    b2L = np.ascontiguousarray(b_ho.reshape(OC, P).T)
    in_maps = []
    for cix in range(NCORES):
        cc = combined[cix * BC:(cix + 1) * BC]  # [BC, K1]
        cL = np.ascontiguousarray(cc.reshape(BC, KO1, P).transpose(2, 1, 0))
        in_maps.append(
            {"c": cL, "w1": w1L, "b1": b1L, "w2": w2L, "b2": b2L}
        )
    return in_maps


def _run(in_maps, **kwargs):
    nc = _build()
    return bass_utils.run_bass_kernel_spmd(
        nc, in_maps, core_ids=list(range(NCORES)), **kwargs
    )


def kernel(x, hidden, W_ih, b_ih, W_ho, b_ho):
    x = np.asarray(x, dtype=np.float32)
    hidden = np.asarray(hidden, dtype=np.float32)
    W_ih = np.asarray(W_ih, dtype=np.float32)
    b_ih = np.asarray(b_ih, dtype=np.float32)
    W_ho = np.asarray(W_ho, dtype=np.float32)
    b_ho = np.asarray(b_ho, dtype=np.float32)

    in_maps = _shard_inputs(x, hidden, W_ih, b_ih, W_ho, b_ho)
    res = _run(in_maps)
    output = np.concatenate([r["outT"].T for r in res.results], axis=0)
    new_hidden = np.concatenate([r["nhT"].T for r in res.results], axis=0)
    return output, new_hidden
